# revision 1
# baseline (speedup 1.0000x reference)
"""Trainium2 Bass kernel for the AdaptiveGaussKronrod VJP quadrature problem.

Math (reference, flattened over N = S*15 = 1920 quadrature nodes):
    phi = sin(t (x) freqs)                  [N, D]
    Z   = phi @ W + b                       [N, D]
    G   = (h*wk)_n * cos(t (x) afreqs) * (1 - tanh(Z)^2)
    out = phi^T @ G                         [D, D]

Sharding: output-column parallel over 8 cores (J = D/8 = 512 columns each).
Core i needs W[:, cols], b[cols], afreqs[cols], full freqs. No collectives:
each core's [D, 512] output block is independent; host concatenates.

Per-core pipeline (Tile framework, bf16 matmuls / fp32 accumulation):
  pass 1 (GEMM1): phi_T tiles ([d, n] layout) generated by ScalarE Sin
    activation in 640-wide n-blocks; Z accumulated in PSUM per n-row-tile;
    epilogue computes G tiles [n, 512] via Tanh / Sin(pi/2 - x) / DVE math.
  pass 2 (GEMM2): phi_N tiles ([n, d] layout) regenerated by ScalarE in
    512-wide d-column blocks (two blocks pre-generated during pass 1);
    out accumulated in PSUM; DMA to DRAM.
All constant broadcast/column tiles are pre-arranged on the host so device
DMAs are contiguous. ScalarE emission interleaves phi generation with the
per-block epilogues so the in-order engine never blocks the PE.
"""

import math

import numpy as np

D = 4096
S = 128
J = D // 8          # output columns per core
N = S * 15          # 1920 quadrature nodes
P = 128
KT = D // P         # 32 k-tiles over D
MT = N // P         # 15 m-tiles over N
OT = D // P         # 32 output row tiles

PT_BLK_M = 5                     # pass-1 n-blocks: 3 x 640 (5 m-tiles each)
PT_BLK_W = PT_BLK_M * P          # 640
PT_NBLK = MT // PT_BLK_M         # 3
PN_BLK_O = 4                     # pass-2 d-col blocks: 8 x 512 (4 o-tiles)
PN_BLK_W = PN_BLK_O * P          # 512
PN_NBLK = OT // PN_BLK_O         # 8

_NODES_NEG = np.array([-0.9914553711208126, -0.9491079123427585, -0.8648644233597691,
                       -0.7415311855993945, -0.5860872354676911, -0.4058451513773972,
                       -0.20778495500789848, 0.0])
_WK_HALF = np.array([0.022935322010529224, 0.06309209262997856, 0.10479001032225019,
                     0.14065325971552592, 0.1690047266392679, 0.19035057806478542,
                     0.20443294007529889, 0.20948214108472782])
GK_NODES = np.concatenate([-_NODES_NEG[:-1][::-1], _NODES_NEG])  # [15]
GK_WK = np.concatenate([_WK_HALF[:-1][::-1], _WK_HALF])          # [15]


def _host_constants():
    edges = np.linspace(0.0, 1.0, S + 1, dtype=np.float32)
    a_s, b_s = edges[:-1], edges[1:]
    h = (b_s - a_s) / 2.0
    c = (a_s + b_s) / 2.0
    t = (c[:, None] + h[:, None] * GK_NODES[None, :].astype(np.float32)).reshape(-1)
    hw = (h[:, None] * GK_WK[None, :].astype(np.float32)).reshape(-1)
    return t.astype(np.float32), hw.astype(np.float32)


def _patch_act_tables():
    """Force Sin AND Tanh to resolve to one table set (silu_and_others) so
    the act-table-load pass emits a single load instead of thrashing
    between trig_and_small and exp_and_others on every Sin<->Tanh switch."""
    import concourse.bacc as bacc_mod
    from concourse import mybir

    if getattr(bacc_mod, "_act_tables_pinned", False):
        return
    orig = bacc_mod.get_activation_tables
    Sin = mybir.ActivationFunctionType.Sin
    Tanh = mybir.ActivationFunctionType.Tanh

    def patched(arch):
        tabs = orig(arch)
        out = {}
        for name, funcs in tabs.items():
            if (Sin in funcs) and (Tanh in funcs):
                out[name] = funcs
            else:
                out[name] = funcs - {Sin, Tanh}
        return out

    bacc_mod.get_activation_tables = patched
    bacc_mod._act_tables_pinned = True


def build_bass():
    """Build and compile the per-core Bass graph (identical on all 8 cores)."""
    from contextlib import ExitStack

    import concourse.bass as bass
    import concourse.tile as tile
    from concourse import bacc, mybir

    _patch_act_tables()

    f32 = mybir.dt.float32
    bf16 = mybir.dt.bfloat16
    Sin = mybir.ActivationFunctionType.Sin
    Tanh = mybir.ActivationFunctionType.Tanh

    nc = bacc.Bacc("TRN2", target_bir_lowering=False, debug=False,
                   enable_asserts=False)

    w_ext = nc.dram_tensor("w", [D, J], f32, kind="ExternalInput")
    tbc_ext = nc.dram_tensor("tbc", [P, N], f32, kind="ExternalInput")
    fbc_ext = nc.dram_tensor("fbc", [P, D], bf16, kind="ExternalInput")
    fpc_ext = nc.dram_tensor("fpc", [P, KT], f32, kind="ExternalInput")
    tpc_ext = nc.dram_tensor("tpc", [P, MT], f32, kind="ExternalInput")
    tnpc_ext = nc.dram_tensor("tnpc", [P, MT], f32, kind="ExternalInput")
    hwpc_ext = nc.dram_tensor("hwpc", [P, MT], f32, kind="ExternalInput")
    afbc_ext = nc.dram_tensor("afbc", [P, J], f32, kind="ExternalInput")
    bbc_ext = nc.dram_tensor("bbc", [P, J], f32, kind="ExternalInput")
    out_ext = nc.dram_tensor("out", [D, J], f32, kind="ExternalOutput")

    with tile.TileContext(nc) as tc, ExitStack() as ctx:
        consts = ctx.enter_context(tc.tile_pool(name="consts", bufs=1))
        stage = ctx.enter_context(tc.tile_pool(name="stage", bufs=3))
        wsp = ctx.enter_context(tc.tile_pool(name="ws", bufs=KT))
        phip = ctx.enter_context(tc.tile_pool(name="phi", bufs=72))
        work = ctx.enter_context(tc.tile_pool(name="work", bufs=2))
        gp = ctx.enter_context(tc.tile_pool(name="g", bufs=MT))
        cp = ctx.enter_context(tc.tile_pool(name="cos", bufs=MT))
        zps = ctx.enter_context(
            tc.tile_pool(name="zpsum", bufs=5, space=bass.MemorySpace.PSUM))
        ops = ctx.enter_context(
            tc.tile_pool(name="opsum", bufs=3, space=bass.MemorySpace.PSUM))

        # ---- PE warm-up: dummy matmuls so HAM reaches K=8/8 before the
        # real GEMM starts (~3.4us of sustained PE activity required) ----
        dummy = consts.tile([P, J], bf16, tag="dummy")
        nc.vector.memset(dummy[:], 0.0)
        wps = ops.tile([P, J], f32, tag="opsum", name="warmps")
        for i in range(64):
            nc.tensor.matmul(wps[:, 0:64], lhsT=dummy[:, 0:128],
                             rhs=dummy[:, 128:192], start=True, stop=True)

        # ---- constants (host-prearranged, contiguous DMAs) ----
        t_bc = consts.tile([P, N], f32, tag="t_bc")
        # chunked so block-0 phi generation starts after the first 640 cols
        for cb in range(PT_NBLK):
            c0 = cb * PT_BLK_W
            nc.sync.dma_start(t_bc[:, c0:c0 + PT_BLK_W],
                              tbc_ext[:, c0:c0 + PT_BLK_W])
        f_pc = consts.tile([P, KT], f32, tag="f_pc")
        nc.sync.dma_start(f_pc[:], fpc_ext[:])
        zero_c = consts.tile([P, 1], f32, tag="zero_c")
        nc.vector.memset(zero_c[:], 0.0)
        halfpi_c = consts.tile([P, 1], f32, tag="halfpi_c")
        nc.vector.memset(halfpi_c[:], math.pi / 2)
        # first ScalarE op: pulls the ACT table load to kernel start
        nc.scalar.activation(halfpi_c[:], zero_c[:], Sin, bias=zero_c[:])
        nc.vector.memset(halfpi_c[:], math.pi / 2)

        # ---- W shard: DMA f32, convert to bf16 (DVE) ----
        ws = []
        for k in range(KT):
            stg = stage.tile([P, J], f32, tag="stage512", name=f"wstg{k}")
            nc.sync.dma_start(stg[:], w_ext[k * P:(k + 1) * P, :])
            wb = wsp.tile([P, J], bf16, tag="ws", name=f"ws{k}")
            nc.vector.tensor_copy(wb[:], stg[:])
            ws.append(wb)

        # remaining constants (needed from the first epilogue onward)
        af_bc = consts.tile([P, J], f32, tag="af_bc")
        nc.sync.dma_start(af_bc[:], afbc_ext[:])
        b_bc = consts.tile([P, J], f32, tag="b_bc")
        nc.sync.dma_start(b_bc[:], bbc_ext[:])
        t_pc = consts.tile([P, MT], f32, tag="t_pc")
        nc.sync.dma_start(t_pc[:], tpc_ext[:])
        tn_pc = consts.tile([P, MT], f32, tag="tn_pc")
        nc.sync.dma_start(tn_pc[:], tnpc_ext[:])
        hw_pc = consts.tile([P, MT], f32, tag="hw_pc")
        nc.sync.dma_start(hw_pc[:], hwpc_ext[:])
        freqs_bc = consts.tile([P, D], bf16, tag="freqs_bc")
        nc.sync.dma_start(freqs_bc[:], fbc_ext[:])

        # ---- builders ----
        def gen_phit_block(blk):
            n0 = blk * PT_BLK_W
            tiles = []
            for k in range(KT):
                pt = phip.tile([P, PT_BLK_W], bf16, tag="phi",
                               name=f"pt{blk}_{k}")
                nc.scalar.activation(pt[:], t_bc[:, n0:n0 + PT_BLK_W], Sin,
                                     bias=zero_c[:], scale=f_pc[:, k:k + 1])
                tiles.append(pt)
            return tiles

        def gen_phin_block(blk):
            c0 = blk * PN_BLK_W
            tiles = []
            for n in range(MT):
                pn = phip.tile([P, PT_BLK_W], bf16, tag="phi",
                               name=f"pn{blk}_{n}")
                nc.scalar.activation(pn[:, :PN_BLK_W],
                                     freqs_bc[:, c0:c0 + PN_BLK_W], Sin,
                                     bias=zero_c[:], scale=t_pc[:, n:n + 1])
                tiles.append(pn)
            return tiles

        def mm_block(blk, phiT, m_outer=False):
            zt = [zps.tile([P, J], f32, tag="zpsum", name=f"zt{blk}_{i}")
                  for i in range(PT_BLK_M)]
            if m_outer:
                # staggered completion: zt[0] finishes a full k-loop early,
                # letting the epilogue produce G tiles while the PE works
                for ml in range(PT_BLK_M):
                    for k in range(KT):
                        nc.tensor.matmul(zt[ml][:],
                                         lhsT=phiT[k][:, ml * P:(ml + 1) * P],
                                         rhs=ws[k][:],
                                         start=(k == 0), stop=(k == KT - 1))
            else:
                for k in range(KT):
                    for ml in range(PT_BLK_M):
                        nc.tensor.matmul(zt[ml][:],
                                         lhsT=phiT[k][:, ml * P:(ml + 1) * P],
                                         rhs=ws[k][:],
                                         start=(k == 0), stop=(k == KT - 1))
            return zt

        def gen_cos_all():
            # cot cosine tiles are GEMM-independent: compute all 15 early
            tiles = []
            for m in range(MT):
                c = cp.tile([P, J], bf16, tag="cos", name=f"cos{m}")
                nc.scalar.activation(c[:], af_bc[:], Sin,
                                     scale=tn_pc[:, m:m + 1], bias=halfpi_c[:])
                tiles.append(c)
            return tiles

        def epilogue(blk, zt):
            # z-adds first: frees all PSUM banks for the next block ASAP
            zs = []
            for ml in range(PT_BLK_M):
                z = work.tile([P, J], f32, tag="z", name=f"z{blk}_{ml}")
                nc.vector.tensor_add(z[:], zt[ml][:], b_bc[:])
                zs.append(z)
            for ml in range(PT_BLK_M):
                m = blk * PT_BLK_M + ml
                z = zs[ml]
                nc.scalar.activation(z[:], z[:], Tanh, bias=zero_c[:])
                s = work.tile([P, J], f32, tag="s", name=f"s{blk}_{ml}")
                nc.vector.tensor_mul(s[:], z[:], z[:])
                nc.vector.tensor_scalar(s[:], s[:], -1.0, 1.0,
                                        mybir.AluOpType.mult, mybir.AluOpType.add)
                v = work.tile([P, J], bf16, tag="v", name=f"v{blk}_{ml}")
                nc.vector.tensor_mul(v[:], cos_tiles[m][:], s[:])
                g = gp.tile([P, J], bf16, tag="g", name=f"g{m}")
                nc.vector.tensor_scalar_mul(g[:], v[:], hw_pc[:, m:m + 1])
                g_tiles[m] = g

        g_tiles = [None] * MT
        # emission order chosen so the in-order ScalarE stream is:
        #   g0 g1 cos | e0 g2 | e1 p2g0 | p2g1 e2 | p2g2 ...
        phiT0 = gen_phit_block(0)
        phiT1 = gen_phit_block(1)
        cos_tiles = gen_cos_all()
        zt0 = mm_block(0, phiT0)
        epilogue(0, zt0)
        phiT2 = gen_phit_block(2)
        zt1 = mm_block(1, phiT1)
        epilogue(1, zt1)
        phiN = {0: gen_phin_block(0)}
        zt2 = mm_block(2, phiT2)
        phiN[1] = gen_phin_block(1)
        epilogue(2, zt2)

        # ---- pass 2: GEMM2 (out = phi^T @ G) ----
        for blk in range(PN_NBLK):
            pn = phiN.pop(blk)
            if blk + 2 < PN_NBLK:
                phiN[blk + 2] = gen_phin_block(blk + 2)
            for ol in range(PN_BLK_O):
                o = blk * PN_BLK_O + ol
                op = ops.tile([P, J], f32, tag="opsum", name=f"op{o}")
                for n in range(MT):
                    nc.tensor.matmul(op[:],
                                     lhsT=pn[n][:, ol * P:(ol + 1) * P],
                                     rhs=g_tiles[n][:],
                                     start=(n == 0), stop=(n == MT - 1))
                ostg = stage.tile([P, J], f32, tag="stage512", name=f"ostg{o}")
                nc.vector.tensor_copy(ostg[:], op[:])
                nc.sync.dma_start(out_ext[o * P:(o + 1) * P, :], ostg[:])

    nc.compile()
    return nc


_CACHE = {}


def _get_nc():
    if "nc" not in _CACHE:
        _CACHE["nc"] = build_bass()
    return _CACHE["nc"]


def kernel(W, b, freqs, afreqs):
    import ml_dtypes
    from concourse.bass_utils import run_bass_kernel_spmd

    W = np.ascontiguousarray(np.asarray(W, dtype=np.float32))
    b = np.asarray(b, dtype=np.float32)
    freqs = np.asarray(freqs, dtype=np.float32)
    afreqs = np.asarray(afreqs, dtype=np.float32)
    t, hw = _host_constants()

    tbc = np.ascontiguousarray(np.broadcast_to(t[None, :], (P, N))).astype(np.float32)
    tpc = np.ascontiguousarray(t.reshape(MT, P).T)
    tnpc = np.ascontiguousarray((-t).reshape(MT, P).T)
    hwpc = np.ascontiguousarray(hw.reshape(MT, P).T)
    fbc = np.ascontiguousarray(
        np.broadcast_to(freqs[None, :], (P, D))).astype(ml_dtypes.bfloat16)
    fpc = np.ascontiguousarray(freqs.reshape(KT, P).T)

    nc = _get_nc()
    in_maps = []
    for i in range(8):
        sl = slice(i * J, (i + 1) * J)
        in_maps.append({
            "w": np.ascontiguousarray(W[:, sl]),
            "tbc": tbc,
            "fbc": fbc,
            "fpc": fpc,
            "tpc": tpc,
            "tnpc": tnpc,
            "hwpc": hwpc,
            "afbc": np.ascontiguousarray(
                np.broadcast_to(afreqs[sl][None, :], (P, J))).astype(np.float32),
            "bbc": np.ascontiguousarray(
                np.broadcast_to(b[sl][None, :], (P, J))).astype(np.float32),
        })
    res = run_bass_kernel_spmd(nc, in_maps, core_ids=list(range(8)))
    return np.concatenate([res.results[i]["out"] for i in range(8)], axis=1)



# revision 2
# speedup vs baseline: 3.3451x; 3.3451x over previous
"""Trainium2 Bass kernel for the AdaptiveGaussKronrod VJP quadrature problem.

Math (reference, flattened over N = S*15 = 1920 quadrature nodes):
    phi = sin(t (x) freqs)                  [N, D]
    Z   = phi @ W + b                       [N, D]
    G   = (h*wk)_n * cos(t (x) afreqs) * (1 - tanh(Z)^2)
    out = phi^T @ G                         [D, D]

Key algebraic optimization: sin(t*f) on t in [0,1], f in [0.5,3] is an
analytic kernel with exponentially decaying singular values — rank 6
reproduces it to ~1e-9 relative. So phi = U @ V^T with U [N,6] (fixed,
depends only on the t nodes) and V [D,6] = v_k(freqs) evaluated on the
host via Chebyshev fits of the right singular functions. This collapses
the two 16-GFLOP GEMMs into rank-6 contractions:
    A   = V^T @ W            [6, J]   (the only pass over W)
    Z   = [U|1] @ [A;b]      [N, J]   (bias via appended ones column)
    G   = cos (.) (1-tanh^2 Z)        (elementwise; cos from ScalarE)
    B   = (U*hw)^T @ G       [6, J]   (hw weights folded into U2)
    out = V @ B              [D, J]
Sharding: output-column parallel over 8 cores (J = D/8 = 512 columns).
No collectives; host concatenates. All matmuls bf16 with fp32 PSUM
accumulation; fro rel err ~3.3e-3 (gate 2e-2).

Per-core timeline: W DMA (bf16, 8x512KB) overlaps A-GEMM + ScalarE cos
generation; middle phase pipelines Z-matmul -> Tanh (ScalarE) ->
y^2/1-q/G (DVE bf16 fast modes) -> B-matmul per 128-node tile; out
phase pipelines V@B matmuls with PSUM->bf16 casts (alternating
ScalarE/DVE) and staged 512KB output DMAs.
"""

import math

import numpy as np

D = 4096
S = 128
NCORES = 8
J = D // NCORES     # output columns per core (512)
N = S * 15          # 1920 quadrature nodes
P = 128
KT = D // P         # 32 k-tiles over D
MT = N // P         # 15 m-tiles over N
OT = D // P         # 32 output row tiles
R = 6               # separable rank of sin(t*f)
KZ = R + 1          # rank rows + ones row (bias)
WCH = 8             # W DMA chunks
WCW = (KT // WCH) * J   # 2048 cols per chunk

_NODES_NEG = np.array([-0.9914553711208126, -0.9491079123427585, -0.8648644233597691,
                       -0.7415311855993945, -0.5860872354676911, -0.4058451513773972,
                       -0.20778495500789848, 0.0])
_WK_HALF = np.array([0.022935322010529224, 0.06309209262997856, 0.10479001032225019,
                     0.14065325971552592, 0.1690047266392679, 0.19035057806478542,
                     0.20443294007529889, 0.20948214108472782])
GK_NODES = np.concatenate([-_NODES_NEG[:-1][::-1], _NODES_NEG])  # [15]
GK_WK = np.concatenate([_WK_HALF[:-1][::-1], _WK_HALF])          # [15]

_FDOM = (0.45, 3.05)    # freq domain covered by the separable basis


def _host_constants():
    edges = np.linspace(0.0, 1.0, S + 1)
    a_s, b_s = edges[:-1], edges[1:]
    h = (b_s - a_s) / 2.0
    c = (a_s + b_s) / 2.0
    t = (c[:, None] + h[:, None] * GK_NODES[None, :]).reshape(-1)
    hw = (h[:, None] * GK_WK[None, :]).reshape(-1)
    return t, hw  # float64 [N]


_FCACHE = {}


def _factorization():
    """U [N,R] on the exact t nodes + Chebyshev fits of the R right
    singular functions v_k(f), so sin(t_i f_j) ~= sum_k U[i,k] v_k(f_j)."""
    if "f" in _FCACHE:
        return _FCACHE["f"]
    t, hw = _host_constants()
    fgrid = np.linspace(_FDOM[0], _FDOM[1], 1200)
    M = np.sin(np.outer(t, fgrid))
    Us, ss, Vt = np.linalg.svd(M, full_matrices=False)
    U = Us[:, :R] * ss[:R]
    chebs = [np.polynomial.chebyshev.Chebyshev.fit(fgrid, Vt[k], 24,
                                                   domain=list(_FDOM))
             for k in range(R)]
    _FCACHE["f"] = (t, hw, U, chebs)
    return _FCACHE["f"]


def _eval_V(freqs):
    t, hw, U, chebs = _factorization()
    f = np.clip(freqs.astype(np.float64), _FDOM[0], _FDOM[1])
    return np.stack([ck(f) for ck in chebs], axis=1)  # [D, R] float64


def _patch_act_tables():
    """Force Sin AND Tanh to resolve to one table set (silu_and_others) so
    the act-table-load pass emits a single load instead of thrashing
    between trig_and_small and exp_and_others on every Sin<->Tanh switch.
    (Copy/Square live in every set, so they are unaffected.)"""
    import concourse.bacc as bacc_mod
    from concourse import mybir

    if getattr(bacc_mod, "_act_tables_pinned", False):
        return
    orig = bacc_mod.get_activation_tables
    Sin = mybir.ActivationFunctionType.Sin
    Tanh = mybir.ActivationFunctionType.Tanh

    def patched(arch):
        tabs = orig(arch)
        out = {}
        for name, funcs in tabs.items():
            if (Sin in funcs) and (Tanh in funcs):
                out[name] = funcs
            else:
                out[name] = funcs - {Sin, Tanh}
        return out

    bacc_mod.get_activation_tables = patched
    bacc_mod._act_tables_pinned = True


def build_bass():
    """Build and compile the per-core Bass graph (identical on all 8 cores)."""
    from contextlib import ExitStack

    import concourse.bass as bass
    import concourse.tile as tile
    from concourse import bacc, mybir

    _patch_act_tables()

    f32 = mybir.dt.float32
    bf16 = mybir.dt.bfloat16
    Sin = mybir.ActivationFunctionType.Sin
    Tanh = mybir.ActivationFunctionType.Tanh
    Copy = mybir.ActivationFunctionType.Copy

    nc = bacc.Bacc("TRN2", target_bir_lowering=False, debug=False,
                   enable_asserts=False)

    wkt_ext = nc.dram_tensor("wkt", [P, KT * J], bf16, kind="ExternalInput")
    utz_ext = nc.dram_tensor("utz", [KZ, N], bf16, kind="ExternalInput")
    vkt_ext = nc.dram_tensor("vkt", [P, KT * R], bf16, kind="ExternalInput")
    u2t_ext = nc.dram_tensor("u2t", [P, MT * R], bf16, kind="ExternalInput")
    vto_ext = nc.dram_tensor("vto", [R, D], bf16, kind="ExternalInput")
    afbc_ext = nc.dram_tensor("afbc", [P, J], f32, kind="ExternalInput")
    tnpc_ext = nc.dram_tensor("tnpc", [P, MT], f32, kind="ExternalInput")
    brow_ext = nc.dram_tensor("brow", [1, J], bf16, kind="ExternalInput")
    out_ext = nc.dram_tensor("out", [P, OT * J], bf16, kind="ExternalOutput")

    with tile.TileContext(nc) as tc, ExitStack() as ctx:
        consts = ctx.enter_context(tc.tile_pool(name="consts", bufs=1))
        wp = ctx.enter_context(tc.tile_pool(name="wp", bufs=3))
        cosp = ctx.enter_context(tc.tile_pool(name="cos", bufs=MT))
        yp = ctx.enter_context(tc.tile_pool(name="y", bufs=3))
        qp = ctx.enter_context(tc.tile_pool(name="q", bufs=3))
        sp = ctx.enter_context(tc.tile_pool(name="s", bufs=3))
        gp = ctx.enter_context(tc.tile_pool(name="g", bufs=3))
        stg = ctx.enter_context(tc.tile_pool(name="stg", bufs=2))
        zps = ctx.enter_context(
            tc.tile_pool(name="zps", bufs=3, space=bass.MemorySpace.PSUM))
        aps = ctx.enter_context(
            tc.tile_pool(name="aps", bufs=1, space=bass.MemorySpace.PSUM))
        bps = ctx.enter_context(
            tc.tile_pool(name="bps", bufs=1, space=bass.MemorySpace.PSUM))
        ops = ctx.enter_context(
            tc.tile_pool(name="ops", bufs=3, space=bass.MemorySpace.PSUM))

        # ---- PE warm-up: dummy matmuls so HAM reaches K=8/8 before the
        # real GEMM stream starts (~3.4us sustained PE activity) ----
        dummy = consts.tile([P, 192], bf16, tag="dummy")
        nc.vector.memset(dummy[:], 0.0)
        wps = ops.tile([P, J], f32, tag="opsum", name="warmps")
        for i in range(56):
            nc.tensor.matmul(wps[:, 0:64], lhsT=dummy[:, 0:128],
                             rhs=dummy[:, 128:192], start=True, stop=True)

        # ---- small constants ----
        zero1 = consts.tile([1, 1], f32, tag="zero1")
        nc.vector.memset(zero1[:], 0.0)
        scr1 = consts.tile([1, 1], f32, tag="scr1")
        # first ScalarE op: pulls the ACT table load to kernel start
        nc.scalar.activation(scr1[:], zero1[:], Sin, bias=0.0)

        afbc = consts.tile([P, J], f32, tag="afbc")
        nc.sync.dma_start(afbc[:], afbc_ext[:])
        tnpc = consts.tile([P, MT], f32, tag="tnpc")
        nc.sync.dma_start(tnpc[:], tnpc_ext[:])
        halfpi = consts.tile([P, 1], f32, tag="halfpi")
        nc.vector.memset(halfpi[:], math.pi / 2)
        utz = consts.tile([KZ, N], bf16, tag="utz")
        nc.sync.dma_start(utz[:], utz_ext[:])
        vkt = consts.tile([P, KT * R], bf16, tag="vkt")
        nc.sync.dma_start(vkt[:], vkt_ext[:])
        u2t = consts.tile([P, MT * R], bf16, tag="u2t")
        nc.sync.dma_start(u2t[:], u2t_ext[:])
        vto = consts.tile([R, D], bf16, tag="vto")
        nc.sync.dma_start(vto[:], vto_ext[:])
        asb = consts.tile([KZ, J], bf16, tag="asb")
        nc.sync.dma_start(asb[R:KZ, :], brow_ext[:])  # bias row
        bsb = consts.tile([R, J], bf16, tag="bsb")

        # ---- W stream + A = V^T W (accumulated over all 32 k-tiles) ----
        wt = []
        for c in range(WCH):
            w = wp.tile([P, WCW], bf16, tag="w", name=f"w{c}")
            nc.sync.dma_start(w[:], wkt_ext[:, c * WCW:(c + 1) * WCW])
            wt.append(w)
        apsum = aps.tile([R, J], f32, tag="apsum")
        for k in range(KT):
            c, kk = divmod(k, KT // WCH)
            nc.tensor.matmul(apsum[:],
                             lhsT=vkt[:, k * R:(k + 1) * R],
                             rhs=wt[c][:, kk * J:(kk + 1) * J],
                             start=(k == 0), stop=(k == KT - 1))

        # ---- cos tiles (Z-independent: fill ScalarE during the W DMA) ----
        cos_tiles = []
        for m in range(MT):
            cm = cosp.tile([P, J], bf16, tag="cos", name=f"cos{m}")
            nc.scalar.activation(cm[:], afbc[:], Sin,
                                 scale=tnpc[:, m:m + 1], bias=halfpi[:])
            cos_tiles.append(cm)

        # A psum -> bf16 rhs rows (bias row already DMA'd)
        nc.vector.tensor_copy(asb[0:R, :], apsum[:])

        # ---- middle: Z -> tanh -> (1-y^2)*cos -> B accumulation ----
        bpsum = bps.tile([R, J], f32, tag="bpsum")
        for m in range(MT):
            zm = zps.tile([P, J], f32, tag="zpsum", name=f"z{m}")
            nc.tensor.matmul(zm[:], lhsT=utz[:, m * P:(m + 1) * P],
                             rhs=asb[:], start=True, stop=True)
            y = yp.tile([P, J], bf16, tag="y", name=f"y{m}")
            nc.scalar.activation(y[:], zm[:], Tanh, bias=0.0)
            q = qp.tile([P, J], bf16, tag="q", name=f"q{m}")
            nc.vector.tensor_mul(q[:], y[:], y[:])
            s = sp.tile([P, J], bf16, tag="s", name=f"s{m}")
            nc.vector.tensor_scalar(s[:], q[:], -1.0, 1.0,
                                    mybir.AluOpType.mult, mybir.AluOpType.add)
            g = gp.tile([P, J], bf16, tag="g", name=f"g{m}")
            nc.vector.tensor_mul(g[:], cos_tiles[m][:], s[:])
            nc.tensor.matmul(bpsum[:], lhsT=u2t[:, m * R:(m + 1) * R],
                             rhs=g[:], start=(m == 0), stop=(m == MT - 1))
        nc.vector.tensor_copy(bsb[:], bpsum[:])

        # ---- out = V @ B, cast to bf16, staged 512KB DMAs ----
        for blk in range(OT // 4):
            st = stg.tile([P, 4 * J], bf16, tag="stage", name=f"st{blk}")
            for ol in range(4):
                o = blk * 4 + ol
                op = ops.tile([P, J], f32, tag="opsum", name=f"op{o}")
                nc.tensor.matmul(op[:], lhsT=vto[:, o * P:(o + 1) * P],
                                 rhs=bsb[:], start=True, stop=True)
                dst = st[:, ol * J:(ol + 1) * J]
                if ol % 2 == 0:
                    nc.scalar.activation(dst, op[:], Copy)
                else:
                    nc.vector.tensor_copy(dst, op[:])
            nc.sync.dma_start(out_ext[:, blk * 4 * J:(blk + 1) * 4 * J], st[:])

    nc.compile()
    return nc


_CACHE = {}


def _get_nc():
    if "nc" not in _CACHE:
        _CACHE["nc"] = build_bass()
    return _CACHE["nc"]


def _in_maps(W, b, freqs, afreqs):
    import ml_dtypes
    bf = ml_dtypes.bfloat16

    t, hw, U, chebs = _factorization()
    V = _eval_V(freqs)                      # [D, R] float64
    U2 = U * hw[:, None]

    utz = np.ones((KZ, N), dtype=np.float32)
    utz[0:R, :] = U.T
    utz = utz.astype(bf)
    vkt = np.ascontiguousarray(
        V.reshape(KT, P, R).transpose(1, 0, 2).reshape(P, KT * R)).astype(bf)
    u2t = np.ascontiguousarray(
        U2.reshape(MT, P, R).transpose(1, 0, 2).reshape(P, MT * R)).astype(bf)
    vto = np.ascontiguousarray(V.T).astype(bf)
    tnpc = np.ascontiguousarray(
        (-t).reshape(MT, P).T).astype(np.float32)

    maps = []
    for i in range(NCORES):
        sl = slice(i * J, (i + 1) * J)
        wkt = np.ascontiguousarray(
            W[:, sl].reshape(KT, P, J).transpose(1, 0, 2).reshape(P, KT * J)
        ).astype(bf)
        maps.append({
            "wkt": wkt,
            "utz": utz,
            "vkt": vkt,
            "u2t": u2t,
            "vto": vto,
            "afbc": np.ascontiguousarray(
                np.broadcast_to(afreqs[sl][None, :], (P, J))).astype(np.float32),
            "tnpc": tnpc,
            "brow": b[sl][None, :].astype(bf),
        })
    return maps


def _assemble(res):
    outs = []
    for i in range(NCORES):
        o = np.asarray(res.results[i]["out"]).astype(np.float32)
        outs.append(o.reshape(P, OT, J).transpose(1, 0, 2).reshape(D, J))
    return np.concatenate(outs, axis=1)


def kernel(W, b, freqs, afreqs):
    from concourse.bass_utils import run_bass_kernel_spmd

    W = np.ascontiguousarray(np.asarray(W, dtype=np.float32))
    b = np.asarray(b, dtype=np.float32)
    freqs = np.asarray(freqs, dtype=np.float32)
    afreqs = np.asarray(afreqs, dtype=np.float32)

    nc = _get_nc()
    maps = _in_maps(W, b, freqs, afreqs)
    res = run_bass_kernel_spmd(nc, maps, core_ids=list(range(NCORES)))
    return _assemble(res)


# revision 5
# speedup vs baseline: 3.3645x; 1.0058x over previous
"""Trainium2 Bass kernel for the AdaptiveGaussKronrod VJP quadrature problem.

Math (reference, flattened over N = S*15 = 1920 quadrature nodes):
    phi = sin(t (x) freqs)                  [N, D]
    Z   = phi @ W + b                       [N, D]
    G   = (h*wk)_n * cos(t (x) afreqs) * (1 - tanh(Z)^2)
    out = phi^T @ G                         [D, D]

Key algebraic optimization: sin(t*f) on t in [0,1], f in [0.5,3] is an
analytic kernel with exponentially decaying singular values — rank 6
reproduces it to ~1e-9 relative. So phi = U @ V^T with U [N,6] (fixed,
depends only on the t nodes) and V [D,6] = v_k(freqs) evaluated on the
host via Chebyshev fits of the right singular functions. This collapses
the two 16-GFLOP GEMMs into rank-6 contractions:
    A   = V^T @ W            [6, J]   (the only pass over W)
    Z   = [U|1] @ [A;b]      [N, J]   (bias via appended ones column)
    G   = cos (.) (1-tanh^2 Z)        (elementwise; cos from ScalarE)
    B   = (U*hw)^T @ G       [6, J]   (hw weights folded into U2)
    out = V @ B              [D, J]
Sharding: output-column parallel over 8 cores (J = D/8 = 512 columns).
No collectives; host concatenates. All matmuls bf16 with fp32 PSUM
accumulation; fro rel err ~3.3e-3 (gate 2e-2).

Per-core timeline: W DMA (bf16, 8x512KB) overlaps A-GEMM + ScalarE cos
generation; middle phase pipelines Z-matmul -> Tanh (ScalarE) ->
y^2/1-q/G (DVE bf16 fast modes) -> B-matmul per 128-node tile; out
phase pipelines V@B matmuls with PSUM->bf16 casts (alternating
ScalarE/DVE) and staged 512KB output DMAs.
"""

import math

import numpy as np

D = 4096
S = 128
NCORES = 8
J = D // NCORES     # output columns per core (512)
N = S * 15          # 1920 quadrature nodes
P = 128
KT = D // P         # 32 k-tiles over D
MT = N // P         # 15 m-tiles over N
OT = D // P         # 32 output row tiles
R = 6               # separable rank of sin(t*f)
KZ = R + 1          # rank rows + ones row (bias)
WCH = 4             # W DMA chunks (1MB each: 8KB per partition per chunk)
WCW = (KT // WCH) * J   # 4096 cols per chunk

_NODES_NEG = np.array([-0.9914553711208126, -0.9491079123427585, -0.8648644233597691,
                       -0.7415311855993945, -0.5860872354676911, -0.4058451513773972,
                       -0.20778495500789848, 0.0])
_WK_HALF = np.array([0.022935322010529224, 0.06309209262997856, 0.10479001032225019,
                     0.14065325971552592, 0.1690047266392679, 0.19035057806478542,
                     0.20443294007529889, 0.20948214108472782])
GK_NODES = np.concatenate([-_NODES_NEG[:-1][::-1], _NODES_NEG])  # [15]
GK_WK = np.concatenate([_WK_HALF[:-1][::-1], _WK_HALF])          # [15]

_FDOM = (0.45, 3.05)    # freq domain covered by the separable basis


def _host_constants():
    edges = np.linspace(0.0, 1.0, S + 1)
    a_s, b_s = edges[:-1], edges[1:]
    h = (b_s - a_s) / 2.0
    c = (a_s + b_s) / 2.0
    t = (c[:, None] + h[:, None] * GK_NODES[None, :]).reshape(-1)
    hw = (h[:, None] * GK_WK[None, :]).reshape(-1)
    return t, hw  # float64 [N]


_FCACHE = {}


def _factorization():
    """U [N,R] on the exact t nodes + Chebyshev fits of the R right
    singular functions v_k(f), so sin(t_i f_j) ~= sum_k U[i,k] v_k(f_j)."""
    if "f" in _FCACHE:
        return _FCACHE["f"]
    t, hw = _host_constants()
    fgrid = np.linspace(_FDOM[0], _FDOM[1], 1200)
    M = np.sin(np.outer(t, fgrid))
    Us, ss, Vt = np.linalg.svd(M, full_matrices=False)
    U = Us[:, :R] * ss[:R]
    chebs = [np.polynomial.chebyshev.Chebyshev.fit(fgrid, Vt[k], 24,
                                                   domain=list(_FDOM))
             for k in range(R)]
    _FCACHE["f"] = (t, hw, U, chebs)
    return _FCACHE["f"]


def _eval_V(freqs):
    t, hw, U, chebs = _factorization()
    f = np.clip(freqs.astype(np.float64), _FDOM[0], _FDOM[1])
    return np.stack([ck(f) for ck in chebs], axis=1)  # [D, R] float64


def _patch_act_tables():
    """Force Sin AND Tanh to resolve to one table set (silu_and_others) so
    the act-table-load pass emits a single load instead of thrashing
    between trig_and_small and exp_and_others on every Sin<->Tanh switch.
    (Copy/Square live in every set, so they are unaffected.)"""
    import concourse.bacc as bacc_mod
    from concourse import mybir

    if getattr(bacc_mod, "_act_tables_pinned", False):
        return
    orig = bacc_mod.get_activation_tables
    Sin = mybir.ActivationFunctionType.Sin
    Tanh = mybir.ActivationFunctionType.Tanh

    def patched(arch):
        tabs = orig(arch)
        out = {}
        for name, funcs in tabs.items():
            if (Sin in funcs) and (Tanh in funcs):
                out[name] = funcs
            else:
                out[name] = funcs - {Sin, Tanh}
        return out

    bacc_mod.get_activation_tables = patched
    bacc_mod._act_tables_pinned = True


def build_bass():
    """Build and compile the per-core Bass graph (identical on all 8 cores)."""
    from contextlib import ExitStack

    import concourse.bass as bass
    import concourse.tile as tile
    from concourse import bacc, mybir

    _patch_act_tables()

    f32 = mybir.dt.float32
    bf16 = mybir.dt.bfloat16
    Sin = mybir.ActivationFunctionType.Sin
    Tanh = mybir.ActivationFunctionType.Tanh
    Copy = mybir.ActivationFunctionType.Copy

    nc = bacc.Bacc("TRN2", target_bir_lowering=False, debug=False,
                   enable_asserts=False)

    wkt_ext = nc.dram_tensor("wkt", [P, KT * J], bf16, kind="ExternalInput")
    utz_ext = nc.dram_tensor("utz", [KZ, N], bf16, kind="ExternalInput")
    vkt_ext = nc.dram_tensor("vkt", [P, KT * R], bf16, kind="ExternalInput")
    u2t_ext = nc.dram_tensor("u2t", [P, MT * R], bf16, kind="ExternalInput")
    vto_ext = nc.dram_tensor("vto", [R, D], bf16, kind="ExternalInput")
    afbc_ext = nc.dram_tensor("afbc", [P, J], f32, kind="ExternalInput")
    tnpc_ext = nc.dram_tensor("tnpc", [P, MT], f32, kind="ExternalInput")
    brow_ext = nc.dram_tensor("brow", [1, J], bf16, kind="ExternalInput")
    out_ext = nc.dram_tensor("out", [P, OT * J], bf16, kind="ExternalOutput")

    NPAIR = MT // 2          # 7 full pairs + 1 trailing single
    with tile.TileContext(nc) as tc, ExitStack() as ctx:
        consts = ctx.enter_context(tc.tile_pool(name="consts", bufs=1))
        wp = ctx.enter_context(tc.tile_pool(name="wp", bufs=3))
        cosp = ctx.enter_context(tc.tile_pool(name="cos", bufs=NPAIR + 1))
        yp = ctx.enter_context(tc.tile_pool(name="y", bufs=2))
        qp = ctx.enter_context(tc.tile_pool(name="q", bufs=2))
        gp = ctx.enter_context(tc.tile_pool(name="g", bufs=2))
        stg = ctx.enter_context(tc.tile_pool(name="stg", bufs=2))
        zps = ctx.enter_context(
            tc.tile_pool(name="zps", bufs=2, space=bass.MemorySpace.PSUM))
        aps = ctx.enter_context(
            tc.tile_pool(name="aps", bufs=1, space=bass.MemorySpace.PSUM))
        bps = ctx.enter_context(
            tc.tile_pool(name="bps", bufs=1, space=bass.MemorySpace.PSUM))
        ops = ctx.enter_context(
            tc.tile_pool(name="ops", bufs=2, space=bass.MemorySpace.PSUM))

        # ---- PE warm-up: dummy matmuls so HAM reaches K=8/8 around the
        # time the first W chunk lands (~3.4us sustained PE activity) ----
        dummy = consts.tile([P, 256], bf16, tag="dummy")
        nc.vector.memset(dummy[:], 0.0)
        wps = ops.tile([P, 2 * J], f32, tag="opsum", name="warmps")
        for i in range(30):
            nc.tensor.matmul(wps[:, 0:128], lhsT=dummy[:, 0:128],
                             rhs=dummy[:, 128:256], start=True, stop=True)

        # ---- W stream (front-loaded: the critical DMA) + lhsT for A ----
        vkt = consts.tile([P, KT * R], bf16, tag="vkt")
        nc.sync.dma_start(vkt[:], vkt_ext[:])
        wt = []
        for c in range(WCH):
            w = wp.tile([P, WCW], bf16, tag="w", name=f"w{c}")
            nc.sync.dma_start(w[:], wkt_ext[:, c * WCW:(c + 1) * WCW])
            wt.append(w)
        apsum = aps.tile([R, J], f32, tag="apsum")
        for k in range(KT):
            c, kk = divmod(k, KT // WCH)
            nc.tensor.matmul(apsum[:],
                             lhsT=vkt[:, k * R:(k + 1) * R],
                             rhs=wt[c][:, kk * J:(kk + 1) * J],
                             start=(k == 0), stop=(k == KT - 1))

        # ---- small constants ----
        zero1 = consts.tile([1, 1], f32, tag="zero1")
        nc.vector.memset(zero1[:], 0.0)
        scr1 = consts.tile([1, 1], f32, tag="scr1")
        # first ScalarE op: pulls the ACT table load to kernel start
        nc.scalar.activation(scr1[:], zero1[:], Sin, bias=0.0)

        afbc = consts.tile([P, J], f32, tag="afbc")
        nc.sync.dma_start(afbc[:], afbc_ext[:])
        tnpc = consts.tile([P, MT], f32, tag="tnpc")
        nc.sync.dma_start(tnpc[:], tnpc_ext[:])
        halfpi = consts.tile([P, 1], f32, tag="halfpi")
        nc.vector.memset(halfpi[:], math.pi / 2)
        utz = consts.tile([KZ, N], bf16, tag="utz")
        nc.sync.dma_start(utz[:], utz_ext[:])
        u2t = consts.tile([P, MT * R], bf16, tag="u2t")
        nc.sync.dma_start(u2t[:], u2t_ext[:])
        vto = consts.tile([R, D], bf16, tag="vto")
        nc.sync.dma_start(vto[:], vto_ext[:])
        asb = consts.tile([KZ, J], bf16, tag="asb")
        nc.sync.dma_start(asb[R:KZ, :], brow_ext[:])  # bias row
        bsb = consts.tile([R, J], bf16, tag="bsb")

        # ---- cos tiles (Z-independent: fill ScalarE during the W DMA);
        # paired [P, 2J] tiles so DVE consumes them in wide ops ----
        cos_tiles = []
        for pr in range((MT + 1) // 2):
            width = 2 * J if 2 * pr + 1 < MT else J
            cm = cosp.tile([P, width], bf16, tag="cos", name=f"cos{pr}")
            for half in range(width // J):
                m = 2 * pr + half
                nc.scalar.activation(cm[:, half * J:(half + 1) * J], afbc[:],
                                     Sin, scale=tnpc[:, m:m + 1], bias=halfpi[:])
            cos_tiles.append(cm)

        # A psum -> bf16 rhs rows (bias row already DMA'd)
        nc.vector.tensor_copy(asb[0:R, :], apsum[:])

        # ---- middle: Z -> tanh -> (y^2-1)*cos -> B accumulation.
        # DVE ops run on [P, 2J] pairs; G = (q-1)*cos is one fused
        # scalar_tensor_tensor (sign absorbed by negated U2). ----
        bpsum = bps.tile([R, J], f32, tag="bpsum")
        for pr in range((MT + 1) // 2):
            width = 2 * J if 2 * pr + 1 < MT else J
            nh = width // J
            yt = yp.tile([P, width], bf16, tag="y", name=f"y{pr}")
            for half in range(nh):
                m = 2 * pr + half
                zm = zps.tile([P, J], f32, tag="zpsum", name=f"z{m}")
                nc.tensor.matmul(zm[:], lhsT=utz[:, m * P:(m + 1) * P],
                                 rhs=asb[:], start=True, stop=True)
                nc.scalar.activation(yt[:, half * J:(half + 1) * J], zm[:],
                                     Tanh, bias=0.0)
            q = qp.tile([P, width], bf16, tag="q", name=f"q{pr}")
            nc.vector.tensor_mul(q[:], yt[:], yt[:])
            g = gp.tile([P, width], bf16, tag="g", name=f"g{pr}")
            nc.vector.scalar_tensor_tensor(g[:], q[:], 1.0, cos_tiles[pr][:],
                                           mybir.AluOpType.subtract,
                                           mybir.AluOpType.mult)
            for half in range(nh):
                m = 2 * pr + half
                nc.tensor.matmul(bpsum[:], lhsT=u2t[:, m * R:(m + 1) * R],
                                 rhs=g[:, half * J:(half + 1) * J],
                                 start=(m == 0), stop=(m == MT - 1))
        nc.vector.tensor_copy(bsb[:], bpsum[:])

        # ---- out = V @ B into paired PSUM, one wide cast per pair
        # (alternating ScalarE/DVE), staged 1MB output DMAs ----
        for blk in range(OT // 8):
            st = stg.tile([P, 8 * J], bf16, tag="stage", name=f"st{blk}")
            for pl in range(4):
                op = ops.tile([P, 2 * J], f32, tag="opsum", name=f"op{blk}_{pl}")
                for half in range(2):
                    o = blk * 8 + pl * 2 + half
                    nc.tensor.matmul(op[:, half * J:(half + 1) * J],
                                     lhsT=vto[:, o * P:(o + 1) * P],
                                     rhs=bsb[:], start=True, stop=True)
                dst = st[:, pl * 2 * J:(pl + 1) * 2 * J]
                if pl % 2 == 0:
                    nc.scalar.activation(dst, op[:], Copy)
                else:
                    nc.vector.tensor_copy(dst, op[:])
            nc.sync.dma_start(out_ext[:, blk * 8 * J:(blk + 1) * 8 * J], st[:])

    nc.compile()
    return nc


_CACHE = {}


def _get_nc():
    if "nc" not in _CACHE:
        _CACHE["nc"] = build_bass()
    return _CACHE["nc"]


def _in_maps(W, b, freqs, afreqs):
    import ml_dtypes
    bf = ml_dtypes.bfloat16

    t, hw, U, chebs = _factorization()
    V = _eval_V(freqs)                      # [D, R] float64
    U2 = -(U * hw[:, None])   # negated: G is computed as (y^2-1)*cos = -G_true

    utz = np.ones((KZ, N), dtype=np.float32)
    utz[0:R, :] = U.T
    utz = utz.astype(bf)
    vkt = np.ascontiguousarray(
        V.reshape(KT, P, R).transpose(1, 0, 2).reshape(P, KT * R)).astype(bf)
    u2t = np.ascontiguousarray(
        U2.reshape(MT, P, R).transpose(1, 0, 2).reshape(P, MT * R)).astype(bf)
    vto = np.ascontiguousarray(V.T).astype(bf)
    tnpc = np.ascontiguousarray(
        (-t).reshape(MT, P).T).astype(np.float32)

    maps = []
    for i in range(NCORES):
        sl = slice(i * J, (i + 1) * J)
        wkt = np.ascontiguousarray(
            W[:, sl].reshape(KT, P, J).transpose(1, 0, 2).reshape(P, KT * J)
        ).astype(bf)
        maps.append({
            "wkt": wkt,
            "utz": utz,
            "vkt": vkt,
            "u2t": u2t,
            "vto": vto,
            "afbc": np.ascontiguousarray(
                np.broadcast_to(afreqs[sl][None, :], (P, J))).astype(np.float32),
            "tnpc": tnpc,
            "brow": b[sl][None, :].astype(bf),
        })
    return maps


def _assemble(res):
    outs = []
    for i in range(NCORES):
        o = np.asarray(res.results[i]["out"]).astype(np.float32)
        outs.append(o.reshape(P, OT, J).transpose(1, 0, 2).reshape(D, J))
    return np.concatenate(outs, axis=1)


def kernel(W, b, freqs, afreqs):
    from concourse.bass_utils import run_bass_kernel_spmd

    W = np.ascontiguousarray(np.asarray(W, dtype=np.float32))
    b = np.asarray(b, dtype=np.float32)
    freqs = np.asarray(freqs, dtype=np.float32)
    afreqs = np.asarray(afreqs, dtype=np.float32)

    nc = _get_nc()
    maps = _in_maps(W, b, freqs, afreqs)
    res = run_bass_kernel_spmd(nc, maps, core_ids=list(range(NCORES)))
    return _assemble(res)


# revision 10
# speedup vs baseline: 3.3977x; 1.0099x over previous
"""Trainium2 Bass kernel for the AdaptiveGaussKronrod VJP quadrature problem.

Math (reference, flattened over N = S*15 = 1920 quadrature nodes):
    phi = sin(t (x) freqs)                  [N, D]
    Z   = phi @ W + b                       [N, D]
    G   = (h*wk)_n * cos(t (x) afreqs) * (1 - tanh(Z)^2)
    out = phi^T @ G                         [D, D]

Key algebraic optimization: sin(t*f) on t in [0,1], f in [0.5,3] is an
analytic kernel with exponentially decaying singular values — rank 6
reproduces it to ~1e-9 relative. So phi = U @ V^T with U [N,6] (fixed,
depends only on the t nodes) and V [D,6] = v_k(freqs) evaluated on the
host via Chebyshev fits of the right singular functions. This collapses
the two 16-GFLOP GEMMs into rank-6 contractions:
    A   = V^T @ W            [6, J]   (the only pass over W)
    Z   = [U|1] @ [A;b]      [N, J]   (bias via appended ones column)
    G   = cos (.) (1-tanh^2 Z)        (elementwise; cos from ScalarE)
    B   = (U*hw)^T @ G       [6, J]   (hw weights folded into U2)
    out = V @ B              [D, J]
Sharding: output-column parallel over 8 cores (J = D/8 = 512 columns).
No collectives; host concatenates. All matmuls bf16 with fp32 PSUM
accumulation; fro rel err ~3.3e-3 (gate 2e-2).

Per-core timeline: W DMA (bf16, 8x512KB) overlaps A-GEMM + ScalarE cos
generation; middle phase pipelines Z-matmul -> Tanh (ScalarE) ->
y^2/1-q/G (DVE bf16 fast modes) -> B-matmul per 128-node tile; out
phase pipelines V@B matmuls with PSUM->bf16 casts (alternating
ScalarE/DVE) and staged 512KB output DMAs.
"""

import math

import numpy as np

D = 4096
S = 128
NCORES = 8
J = D // NCORES     # output columns per core (512)
N = S * 15          # 1920 quadrature nodes
P = 128
KT = D // P         # 32 k-tiles over D
MT = N // P         # 15 m-tiles over N
OT = D // P         # 32 output row tiles
R = 6               # separable rank of sin(t*f)
KZ = R + 1          # rank rows + ones row (bias)
WCH = 4             # W DMA chunks (1MB each: 8KB per partition per chunk)
WCW = (KT // WCH) * J   # 4096 cols per chunk

_NODES_NEG = np.array([-0.9914553711208126, -0.9491079123427585, -0.8648644233597691,
                       -0.7415311855993945, -0.5860872354676911, -0.4058451513773972,
                       -0.20778495500789848, 0.0])
_WK_HALF = np.array([0.022935322010529224, 0.06309209262997856, 0.10479001032225019,
                     0.14065325971552592, 0.1690047266392679, 0.19035057806478542,
                     0.20443294007529889, 0.20948214108472782])
GK_NODES = np.concatenate([-_NODES_NEG[:-1][::-1], _NODES_NEG])  # [15]
GK_WK = np.concatenate([_WK_HALF[:-1][::-1], _WK_HALF])          # [15]

_FDOM = (0.45, 3.05)    # freq domain covered by the separable basis


def _host_constants():
    edges = np.linspace(0.0, 1.0, S + 1)
    a_s, b_s = edges[:-1], edges[1:]
    h = (b_s - a_s) / 2.0
    c = (a_s + b_s) / 2.0
    t = (c[:, None] + h[:, None] * GK_NODES[None, :]).reshape(-1)
    hw = (h[:, None] * GK_WK[None, :]).reshape(-1)
    return t, hw  # float64 [N]


_FCACHE = {}


def _factorization():
    """U [N,R] on the exact t nodes + Chebyshev fits of the R right
    singular functions v_k(f), so sin(t_i f_j) ~= sum_k U[i,k] v_k(f_j)."""
    if "f" in _FCACHE:
        return _FCACHE["f"]
    t, hw = _host_constants()
    fgrid = np.linspace(_FDOM[0], _FDOM[1], 1200)
    M = np.sin(np.outer(t, fgrid))
    Us, ss, Vt = np.linalg.svd(M, full_matrices=False)
    U = Us[:, :R] * ss[:R]
    chebs = [np.polynomial.chebyshev.Chebyshev.fit(fgrid, Vt[k], 24,
                                                   domain=list(_FDOM))
             for k in range(R)]
    _FCACHE["f"] = (t, hw, U, chebs)
    return _FCACHE["f"]


def _eval_V(freqs):
    t, hw, U, chebs = _factorization()
    f = np.clip(freqs.astype(np.float64), _FDOM[0], _FDOM[1])
    return np.stack([ck(f) for ck in chebs], axis=1)  # [D, R] float64


def _patch_act_tables():
    """Force Sin AND Tanh to resolve to one table set (silu_and_others) so
    the act-table-load pass emits a single load instead of thrashing
    between trig_and_small and exp_and_others on every Sin<->Tanh switch.
    (Copy/Square live in every set, so they are unaffected.)"""
    import concourse.bacc as bacc_mod
    from concourse import mybir

    if getattr(bacc_mod, "_act_tables_pinned", False):
        return
    orig = bacc_mod.get_activation_tables
    Sin = mybir.ActivationFunctionType.Sin
    Tanh = mybir.ActivationFunctionType.Tanh

    def patched(arch):
        tabs = orig(arch)
        out = {}
        for name, funcs in tabs.items():
            if (Sin in funcs) and (Tanh in funcs):
                out[name] = funcs
            else:
                out[name] = funcs - {Sin, Tanh}
        return out

    bacc_mod.get_activation_tables = patched
    bacc_mod._act_tables_pinned = True


def build_bass():
    """Build and compile the per-core Bass graph (identical on all 8 cores)."""
    from contextlib import ExitStack

    import concourse.bass as bass
    import concourse.tile as tile
    from concourse import bacc, mybir

    _patch_act_tables()

    f32 = mybir.dt.float32
    bf16 = mybir.dt.bfloat16
    Sin = mybir.ActivationFunctionType.Sin
    Tanh = mybir.ActivationFunctionType.Tanh
    Copy = mybir.ActivationFunctionType.Copy

    nc = bacc.Bacc("TRN2", target_bir_lowering=False, debug=False,
                   enable_asserts=False)

    wkt_ext = nc.dram_tensor("wkt", [P, KT * J], bf16, kind="ExternalInput")
    utz_ext = nc.dram_tensor("utz", [KZ, N], bf16, kind="ExternalInput")
    vkt_ext = nc.dram_tensor("vkt", [P, KT * R], bf16, kind="ExternalInput")
    u2t_ext = nc.dram_tensor("u2t", [P, MT * 2 * R], bf16, kind="ExternalInput")
    vto_ext = nc.dram_tensor("vto", [R, D], bf16, kind="ExternalInput")
    afbc_ext = nc.dram_tensor("afbc", [P, J], f32, kind="ExternalInput")
    tnpc_ext = nc.dram_tensor("tnpc", [P, MT], f32, kind="ExternalInput")
    brow_ext = nc.dram_tensor("brow", [1, J], bf16, kind="ExternalInput")
    out_ext = nc.dram_tensor("out", [P, OT * J], bf16, kind="ExternalOutput")

    NPAIR = MT // 2          # 7 full pairs + 1 trailing single
    with tile.TileContext(nc) as tc, ExitStack() as ctx:
        consts = ctx.enter_context(tc.tile_pool(name="consts", bufs=1))
        wp = ctx.enter_context(tc.tile_pool(name="wp", bufs=3))
        cosp = ctx.enter_context(tc.tile_pool(name="cos", bufs=NPAIR + 1))
        yp = ctx.enter_context(tc.tile_pool(name="y", bufs=2))
        qp = ctx.enter_context(tc.tile_pool(name="q", bufs=2))
        gp = ctx.enter_context(tc.tile_pool(name="g", bufs=2))
        stg = ctx.enter_context(tc.tile_pool(name="stg", bufs=2))
        zps = ctx.enter_context(
            tc.tile_pool(name="zps", bufs=2, space=bass.MemorySpace.PSUM))
        aps = ctx.enter_context(
            tc.tile_pool(name="aps", bufs=1, space=bass.MemorySpace.PSUM))
        bps = ctx.enter_context(
            tc.tile_pool(name="bps", bufs=1, space=bass.MemorySpace.PSUM))
        ops = ctx.enter_context(
            tc.tile_pool(name="ops", bufs=2, space=bass.MemorySpace.PSUM))

        # ---- PE warm-up: dummy matmuls so HAM reaches K=8/8 around the
        # time the first W chunk lands (needs a FULL 3.4us busy window) ----
        dummy = consts.tile([P, 256], bf16, tag="dummy")
        nc.vector.memset(dummy[:], 0.0)
        wps = ops.tile([P, 2 * J], f32, tag="opsum", name="warmps")
        for i in range(42):
            nc.tensor.matmul(wps[:, 0:128], lhsT=dummy[:, 0:128],
                             rhs=dummy[:, 128:256], start=True, stop=True)

        # ---- small const DMAs FIRST (the Sync queue is in-order: these
        # must not trail the 11us W stream) ----
        vkt = consts.tile([P, KT * R], bf16, tag="vkt")
        nc.sync.dma_start(vkt[:], vkt_ext[:])
        afbc = consts.tile([P, J], f32, tag="afbc")
        nc.sync.dma_start(afbc[:], afbc_ext[:])
        tnpc = consts.tile([P, MT], f32, tag="tnpc")
        nc.sync.dma_start(tnpc[:], tnpc_ext[:])
        utz = consts.tile([KZ, N], bf16, tag="utz")
        nc.sync.dma_start(utz[:], utz_ext[:])
        u2t = consts.tile([P, MT * 2 * R], bf16, tag="u2t")
        nc.sync.dma_start(u2t[:], u2t_ext[:])
        vto = consts.tile([R, D], bf16, tag="vto")
        nc.sync.dma_start(vto[:], vto_ext[:])
        asb = consts.tile([KZ, J], bf16, tag="asb")
        nc.sync.dma_start(asb[R:KZ, :], brow_ext[:])  # bias row
        bsb = consts.tile([R, J], bf16, tag="bsb")

        # ---- W stream + A = V^T W (accumulated over all 32 k-tiles) ----
        wt = []
        for c in range(WCH):
            w = wp.tile([P, WCW], bf16, tag="w", name=f"w{c}")
            nc.sync.dma_start(w[:], wkt_ext[:, c * WCW:(c + 1) * WCW])
            wt.append(w)
        apsum = aps.tile([R, J], f32, tag="apsum")
        for k in range(KT):
            c, kk = divmod(k, KT // WCH)
            nc.tensor.matmul(apsum[:],
                             lhsT=vkt[:, k * R:(k + 1) * R],
                             rhs=wt[c][:, kk * J:(kk + 1) * J],
                             start=(k == 0), stop=(k == KT - 1))

        # ---- small non-DMA constants ----
        zero1 = consts.tile([1, 1], f32, tag="zero1")
        nc.vector.memset(zero1[:], 0.0)
        scr1 = consts.tile([1, 1], f32, tag="scr1")
        # first ScalarE op: pulls the ACT table load to kernel start
        nc.scalar.activation(scr1[:], zero1[:], Sin, bias=0.0)
        halfpi = consts.tile([P, 1], f32, tag="halfpi")
        nc.vector.memset(halfpi[:], math.pi / 2)

        # ---- cos tiles (Z-independent: fill ScalarE during the W DMA);
        # paired [P, 2J] tiles so DVE consumes them in wide ops ----
        cos_tiles = []
        for pr in range((MT + 1) // 2):
            width = 2 * J if 2 * pr + 1 < MT else J
            cm = cosp.tile([P, width], bf16, tag="cos", name=f"cos{pr}")
            for half in range(width // J):
                m = 2 * pr + half
                nc.scalar.activation(cm[:, half * J:(half + 1) * J], afbc[:],
                                     Sin, scale=tnpc[:, m:m + 1], bias=halfpi[:])
            cos_tiles.append(cm)

        # A psum -> bf16 rhs rows (bias row already DMA'd)
        nc.vector.tensor_copy(asb[0:R, :], apsum[:])

        # ---- B = U2^T (cos (.) (1-y^2)) distributed as
        #      B = (+U2)^T cos + (-U2)^T (cos*y*y)
        # The cos-term matmuls need only the cos tiles, so they run during
        # the W DMA; the elementwise part is just two chained TTs per pair
        # (u = cos*y, v = u*y) in DVE's bf16 2x mode. u2t holds +U2 at
        # block 2m and -U2 at block 2m+1. ----
        bpsum = bps.tile([R, J], f32, tag="bpsum")
        for m in range(MT):
            pr, half = divmod(m, 2)
            nc.tensor.matmul(bpsum[:], lhsT=u2t[:, 2 * m * R:(2 * m + 1) * R],
                             rhs=cos_tiles[pr][:, half * J:(half + 1) * J],
                             start=(m == 0), stop=False)

        # ---- middle: Z -> tanh -> u=cos*y, v=u*y -> -U2 B accumulation ----
        for pr in range((MT + 1) // 2):
            width = 2 * J if 2 * pr + 1 < MT else J
            nh = width // J
            yt = yp.tile([P, width], bf16, tag="y", name=f"y{pr}")
            for half in range(nh):
                m = 2 * pr + half
                zm = zps.tile([P, J], f32, tag="zpsum", name=f"z{m}")
                nc.tensor.matmul(zm[:], lhsT=utz[:, m * P:(m + 1) * P],
                                 rhs=asb[:], start=True, stop=True)
                nc.scalar.activation(yt[:, half * J:(half + 1) * J], zm[:],
                                     Tanh, bias=0.0)
            u = qp.tile([P, width], bf16, tag="q", name=f"u{pr}")
            nc.vector.tensor_mul(u[:], cos_tiles[pr][:, 0:width], yt[:])
            v = gp.tile([P, width], bf16, tag="g", name=f"v{pr}")
            nc.vector.tensor_mul(v[:], u[:], yt[:])
            for half in range(nh):
                m = 2 * pr + half
                nc.tensor.matmul(bpsum[:],
                                 lhsT=u2t[:, (2 * m + 1) * R:(2 * m + 2) * R],
                                 rhs=v[:, half * J:(half + 1) * J],
                                 start=False, stop=(m == MT - 1))
        nc.vector.tensor_copy(bsb[:], bpsum[:])

        # ---- out = V @ B into paired PSUM, one wide cast per pair
        # (alternating ScalarE/DVE), staged 1MB output DMAs ----
        for blk in range(OT // 8):
            st = stg.tile([P, 8 * J], bf16, tag="stage", name=f"st{blk}")
            for pl in range(4):
                op = ops.tile([P, 2 * J], f32, tag="opsum", name=f"op{blk}_{pl}")
                for half in range(2):
                    o = blk * 8 + pl * 2 + half
                    nc.tensor.matmul(op[:, half * J:(half + 1) * J],
                                     lhsT=vto[:, o * P:(o + 1) * P],
                                     rhs=bsb[:], start=True, stop=True)
                dst = st[:, pl * 2 * J:(pl + 1) * 2 * J]
                if pl % 2 == 0:
                    nc.scalar.activation(dst, op[:], Copy)
                else:
                    nc.vector.tensor_copy(dst, op[:])
            nc.sync.dma_start(out_ext[:, blk * 8 * J:(blk + 1) * 8 * J], st[:])

    nc.compile()
    return nc


_CACHE = {}


def _get_nc():
    if "nc" not in _CACHE:
        _CACHE["nc"] = build_bass()
    return _CACHE["nc"]


def _in_maps(W, b, freqs, afreqs):
    import ml_dtypes
    bf = ml_dtypes.bfloat16

    t, hw, U, chebs = _factorization()
    V = _eval_V(freqs)                      # [D, R] float64
    U2 = U * hw[:, None]

    utz = np.ones((KZ, N), dtype=np.float32)
    utz[0:R, :] = U.T
    utz = utz.astype(bf)
    vkt = np.ascontiguousarray(
        V.reshape(KT, P, R).transpose(1, 0, 2).reshape(P, KT * R)).astype(bf)
    # interleaved per m-tile: block 2m = +U2 (cos term), 2m+1 = -U2 (cos*y^2)
    u2pm = np.stack([U2.reshape(MT, P, R), -U2.reshape(MT, P, R)],
                    axis=1)                              # [MT, 2, P, R]
    u2t = np.ascontiguousarray(
        u2pm.transpose(2, 0, 1, 3).reshape(P, MT * 2 * R)).astype(bf)
    vto = np.ascontiguousarray(V.T).astype(bf)
    tnpc = np.ascontiguousarray(
        (-t).reshape(MT, P).T).astype(np.float32)

    maps = []
    for i in range(NCORES):
        sl = slice(i * J, (i + 1) * J)
        wkt = np.ascontiguousarray(
            W[:, sl].reshape(KT, P, J).transpose(1, 0, 2).reshape(P, KT * J)
        ).astype(bf)
        maps.append({
            "wkt": wkt,
            "utz": utz,
            "vkt": vkt,
            "u2t": u2t,
            "vto": vto,
            "afbc": np.ascontiguousarray(
                np.broadcast_to(afreqs[sl][None, :], (P, J))).astype(np.float32),
            "tnpc": tnpc,
            "brow": b[sl][None, :].astype(bf),
        })
    return maps


def _assemble(res):
    outs = []
    for i in range(NCORES):
        o = np.asarray(res.results[i]["out"]).astype(np.float32)
        outs.append(o.reshape(P, OT, J).transpose(1, 0, 2).reshape(D, J))
    return np.concatenate(outs, axis=1)


def kernel(W, b, freqs, afreqs):
    from concourse.bass_utils import run_bass_kernel_spmd

    W = np.ascontiguousarray(np.asarray(W, dtype=np.float32))
    b = np.asarray(b, dtype=np.float32)
    freqs = np.asarray(freqs, dtype=np.float32)
    afreqs = np.asarray(afreqs, dtype=np.float32)

    nc = _get_nc()
    maps = _in_maps(W, b, freqs, afreqs)
    res = run_bass_kernel_spmd(nc, maps, core_ids=list(range(NCORES)))
    return _assemble(res)
